# revision 6
# baseline (speedup 1.0000x reference)
"""Sparse GQA attention (causal + sliding window + global tokens) with LoRA
projections and RoPE, distributed over 8 TRN2 NeuronCores.

Sharding: batch (2) x kv-head-group (4). Core (b, g) computes q heads
4g..4g+3 and kv head g for batch b, producing a partial output-projection
sum; the host adds the 4 group partials per batch.

Host-side preprocessing (exact, linear):
  - LoRA folded into the dense weights: W_eff = W + B @ A.
  - Weights pre-transposed into matmul (lhsT / rhs) layouts, cast to bf16.
  - q/k weight rows permuted to the rotate-half layout (evens then odds)
    so RoPE becomes half-tile multiplies.
  - x transposed to [D, S] per batch (contraction dim on partitions).

Kernel structure (per core):
  Phase A (per 512-token chunk): QKV projections; RoPE on a bf16 SBUF
  copy of the projection PSUM; V transposed to natural [tok, hd] layout
  with a ones column appended (for softmax sums via matmul).
  Phase B (per 4-query-block quarter, interleaved with A): scores are
  computed TRANSPOSED (s^T[k, q]) batched per key-block covering its
  whole 5-query-block range in wide matmuls; exp on scalar engine;
  causal/window masking as post-exp {0,1} multiplies on DVE; P@V with
  the ones column yielding row sums in PSUM; normalization fused into
  the PSUM->SBUF copy; o transposed via one 128-col matmul; output
  projection per query block.
"""

import os
import sys

import numpy as np

for _p in ("/root/.axon_site", "/root/.axon_site/_ro/trn_rl_repo",
           "/root/.axon_site/_ro/pypackages", "/opt/trn_rl_repo"):
    if os.path.isdir(_p) and _p not in sys.path:
        sys.path.append(_p)

import ml_dtypes
import concourse.bacc as bacc
import concourse.mybir as mybir
import concourse.tile as tile
from concourse.bass_utils import run_bass_kernel_spmd

B, S, D = 2, 2048, 2048
H, KVH, HD = 16, 4, 128
WINDOW, GLOBAL = 512, 64
THETA = 1000000.0
NCORES = 8
GH = H // KVH          # q heads per core
GF = GH * HD           # 512 projection features per core
TOK = 512              # token chunk for projections
NCH = S // TOK         # 4 chunks
NQB = S // 128         # 16 query blocks
NKB = S // 128
INV_SQRT = 1.0 / float(np.sqrt(HD))

F32 = mybir.dt.float32
BF16 = mybir.dt.bfloat16

_PROGRAM = {}


def _emit(nc, t):
    """Emit the per-core Tile program. `t` maps input names to DRAM APs."""
    tc = t["tc"]
    from contextlib import ExitStack

    xTd = t["xt"].ap()      # [NCH,128,16,TOK] chunk-packed
    wqT = t["wqt"].ap()     # [128,16,GF]
    wkT = t["wkt"].ap()     # [128,16,HD]
    wvT = t["wvt"].ap()
    woT = t["wot"].ap()     # [128,GH,D]
    y = t["y"].ap()         # [S, D]

    with ExitStack() as stk:
        singles = stk.enter_context(tc.tile_pool(name="singles", bufs=1))
        persist = stk.enter_context(tc.tile_pool(name="persist", bufs=1))
        wpool = stk.enter_context(tc.tile_pool(name="wpool", bufs=1))
        xpool = stk.enter_context(tc.tile_pool(name="xpool", bufs=2))
        apool = stk.enter_context(tc.tile_pool(name="apool", bufs=2))
        bpool = stk.enter_context(tc.tile_pool(name="bpool", bufs=1))
        spool = stk.enter_context(tc.tile_pool(name="spool", bufs=4))
        cpool = stk.enter_context(tc.tile_pool(name="cpool", bufs=2))
        psa = stk.enter_context(tc.tile_pool(name="psa", bufs=1,
                                             space="PSUM"))
        psb = stk.enter_context(tc.tile_pool(name="psb", bufs=1,
                                             space="PSUM"))

        ident_sb = singles.tile([128, 128], BF16)
        nc.sync.dma_start(out=ident_sb, in_=t["ident"].ap())
        triT_sb = singles.tile([128, 128], BF16)
        nc.sync.dma_start(out=triT_sb, in_=t["triT"].ap())
        edgeT_sb = singles.tile([128, 128], BF16)
        nc.sync.dma_start(out=edgeT_sb, in_=t["edgeT"].ap())
        edgegT_sb = singles.tile([128, 128], BF16)
        nc.sync.dma_start(out=edgegT_sb, in_=t["edgegT"].ap())

        qT_sb = persist.tile([128, GH, S], BF16)    # [hd, head, tok]
        kT_sb = persist.tile([128, S], BF16)        # [hd, tok]
        v_sb = persist.tile([128, NKB, HD + 1], BF16)  # [tok%128, kblk, hd|1]
        nc.vector.memset(v_sb[:, :, HD:HD + 1], 1.0)

        wq_sb = wpool.tile([128, 16, GF], BF16)
        nc.sync.dma_start(out=wq_sb, in_=wqT)
        wk_sb = wpool.tile([128, 16, HD], BF16)
        nc.sync.dma_start(out=wk_sb, in_=wkT)
        wv_sb = wpool.tile([128, 16, HD], BF16)
        nc.sync.dma_start(out=wv_sb, in_=wvT)
        cos_sb = wpool.tile([128, S], BF16)
        nc.sync.dma_start(out=cos_sb, in_=t["cos2t"].ap())
        sin_sb = wpool.tile([128, S], BF16)
        nc.sync.dma_start(out=sin_sb, in_=t["sins2t"].ap())
        wo_sb = wpool.tile([128, GH, D], BF16)

        ptk_tiles = {}
        ptg_tiles = {}

        def emit_chunk(c):
            cs = slice(c * TOK, (c + 1) * TOK)
            xt = xpool.tile([128, 16, TOK], BF16, tag="xt")
            nc.sync.dma_start(out=xt, in_=xTd[c])
            # q (GH heads) and k (1 head) with rotate-half RoPE in bf16
            for h in range(GH + 1):
                if h < GH:
                    wslc = wq_sb[:, :, h * HD:(h + 1) * HD]
                    dst = qT_sb[:, h, cs]
                else:
                    wslc = wk_sb
                    dst = kT_sb[:, cs]
                pq = psa.tile([128, TOK], F32, tag="pq", bufs=2)
                for a in range(16):
                    nc.tensor.matmul(pq, wslc[:, a, :], xt[:, a, :],
                                     start=(a == 0), stop=(a == 15))
                pqs = apool.tile([128, TOK], BF16, tag="pqs", bufs=2)
                nc.scalar.copy(pqs, pq)
                # rotate-half copy (cross-partition reads allowed from PSUM)
                pqr = apool.tile([128, TOK], BF16, tag="pqr", bufs=2)
                nc.scalar.copy(pqr[0:64, :], pq[64:128, :])
                nc.scalar.copy(pqr[64:128, :], pq[0:64, :])
                t1 = apool.tile([128, TOK], BF16, tag="t1", bufs=2)
                nc.vector.tensor_mul(t1, pqr, sin_sb[:, cs])
                t2 = apool.tile([128, TOK], BF16, tag="t2", bufs=2)
                nc.vector.tensor_mul(t2, pqs, cos_sb[:, cs])
                nc.vector.tensor_add(dst, t2, t1)
            # v: compute vT then transpose to natural [tok, hd] bf16
            pv = psa.tile([128, TOK], F32, tag="pq", bufs=2)
            for a in range(16):
                nc.tensor.matmul(pv, wv_sb[:, a, :], xt[:, a, :],
                                 start=(a == 0), stop=(a == 15))
            vt = apool.tile([128, TOK], BF16, tag="vt", bufs=2)
            nc.scalar.copy(vt, pv)
            vtp = psa.tile([128, 4, 128], BF16, tag="vtp", bufs=1)
            for b2 in range(TOK // 128):
                nc.tensor.transpose(vtp[:, b2, :],
                                    vt[:, b2 * 128:(b2 + 1) * 128],
                                    ident_sb)
            nc.vector.tensor_copy(v_sb[:, c * 4:(c + 1) * 4, 0:HD], vtp)

        def emit_quarter(qtr):
            qis = list(range(4 * (qtr - 1), 4 * qtr))
            for h in range(GH):
                for kb in qis:
                    # score tile kb covers q-blocks kb..kb+3 (diag at j=0);
                    # the j=4 window-edge block is computed per-qi below.
                    nq = min(4, NQB - kb)
                    W = 128 * nq
                    k0 = kb * 128
                    ps = psb.tile([128, 512], F32, tag="ps", bufs=2)
                    nc.tensor.matmul(ps[:, 0:W], kT_sb[:, k0:k0 + 128],
                                     qT_sb[:, h, k0:k0 + W],
                                     start=True, stop=True)
                    ptk = bpool.tile([128, 512], BF16, tag=f"ptk{h}",
                                     bufs=8)
                    nc.scalar.activation(ptk[:, 0:W], ps[:, 0:W],
                                         mybir.ActivationFunctionType.Exp,
                                         scale=INV_SQRT)
                    nc.vector.tensor_mul(ptk[:, 0:128], ptk[:, 0:128],
                                         triT_sb)
                    ptk_tiles[(h, kb)] = ptk
                if qtr >= 2:
                    q0b = max(5, 4 * (qtr - 1))
                    Wg = (4 * qtr - q0b) * 128
                    q0 = q0b * 128
                    gps = psb.tile([128, 512], F32, tag="ps", bufs=2)
                    nc.tensor.matmul(gps[0:64, 0:Wg], kT_sb[:, 0:64],
                                     qT_sb[:, h, q0:q0 + Wg],
                                     start=True, stop=True)
                    ptg = bpool.tile([128, 512], BF16, tag=f"ptg{h}",
                                     bufs=2)
                    nc.scalar.activation(ptg[0:64, 0:Wg], gps[0:64, 0:Wg],
                                         mybir.ActivationFunctionType.Exp,
                                         scale=INV_SQRT)
                    ptg_tiles[h] = (ptg, q0)
            for qi in qis:
                ot = cpool.tile([128, GH, 128], BF16, tag="ot", bufs=2)
                for h in range(GH):
                    # po tile layout: [0:129] P@V+sums accum, [132:260]
                    # o-transpose out, [384:512] window-edge scores.
                    po = psb.tile([128, 512], F32, tag="po", bufs=2)
                    kbs = list(range(max(0, qi - 4), qi + 1))
                    n_mm = len(kbs) + (1 if qi >= 5 else 0)
                    if qi >= 4:
                        ke = (qi - 4) * 128
                        nc.tensor.matmul(po[:, 384:512],
                                         kT_sb[:, ke:ke + 128],
                                         qT_sb[:, h, qi * 128:qi * 128 + 128],
                                         start=True, stop=True)
                        pedge = spool.tile([128, 128], BF16, tag="pedge")
                        nc.scalar.activation(
                            pedge, po[:, 384:512],
                            mybir.ActivationFunctionType.Exp,
                            scale=INV_SQRT)
                        nc.vector.tensor_mul(
                            pedge, pedge,
                            edgegT_sb if qi == 4 else edgeT_sb)
                    # window blocks first, edge (j=4) last: hides the
                    # edge exp+mask latency under the window PV matmuls
                    kbs = kbs[::-1] if len(kbs) == 5 else kbs
                    for i, kb in enumerate(kbs):
                        j = qi - kb
                        lhsT = (pedge if j == 4 else
                                ptk_tiles[(h, kb)][:, j * 128:(j + 1) * 128])
                        nc.tensor.matmul(
                            po[:, 0:HD + 1], lhsT, v_sb[:, kb, 0:HD + 1],
                            start=(i == 0), stop=(i == n_mm - 1))
                    if qi >= 5:
                        ptg, q0 = ptg_tiles[h]
                        off = qi * 128 - q0
                        nc.tensor.matmul(po[:, 0:HD + 1],
                                         ptg[0:64, off:off + 128],
                                         v_sb[0:64, 0, 0:HD + 1],
                                         start=False, stop=True)
                    inv = spool.tile([128, 1], F32, tag="inv")
                    nc.vector.reciprocal(inv, po[:, HD:HD + 1])
                    onat = spool.tile([128, 128], BF16, tag="onat")
                    nc.vector.tensor_scalar_mul(onat, po[:, 0:HD], inv)
                    nc.tensor.matmul(po[:, 132:260], onat, ident_sb,
                                     start=True, stop=True)
                    nc.scalar.copy(ot[:, h, :], po[:, 132:260])
                # output projection for this token block
                ysb = cpool.tile([128, D], BF16, tag="ysb", bufs=2)
                for cchunk in range(4):
                    ns = slice(cchunk * 512, (cchunk + 1) * 512)
                    py = psb.tile([128, 512], F32, tag="py", bufs=1)
                    for hh in range(GH):
                        nc.tensor.matmul(py, ot[:, hh, :],
                                         wo_sb[:, hh, ns],
                                         start=(hh == 0),
                                         stop=(hh == GH - 1))
                    if cchunk % 2 == 0:
                        nc.scalar.copy(ysb[:, ns], py)
                    else:
                        nc.vector.tensor_copy(ysb[:, ns], py)
                nc.sync.dma_start(out=y[qi * 128:(qi + 1) * 128, :],
                                  in_=ysb)

        emit_chunk(0)
        nc.sync.dma_start(out=wo_sb, in_=woT)
        emit_chunk(1)
        emit_quarter(1)
        emit_chunk(2)
        emit_quarter(2)
        emit_chunk(3)
        emit_quarter(3)
        emit_quarter(4)


def _build_program():
    if "nc" in _PROGRAM:
        return _PROGRAM["nc"]
    nc = bacc.Bacc("TRN2", target_bir_lowering=False, debug=False,
                   num_devices=NCORES)
    t = {}
    t["xt"] = nc.dram_tensor("xt", [NCH, 128, 16, TOK], BF16,
                             kind="ExternalInput")
    t["wqt"] = nc.dram_tensor("wqt", [128, 16, GF], BF16,
                              kind="ExternalInput")
    t["wkt"] = nc.dram_tensor("wkt", [128, 16, HD], BF16,
                              kind="ExternalInput")
    t["wvt"] = nc.dram_tensor("wvt", [128, 16, HD], BF16,
                              kind="ExternalInput")
    t["wot"] = nc.dram_tensor("wot", [128, GH, D], BF16,
                              kind="ExternalInput")
    t["cos2t"] = nc.dram_tensor("cos2t", [128, S], BF16,
                                kind="ExternalInput")
    t["sins2t"] = nc.dram_tensor("sins2t", [128, S], BF16,
                                 kind="ExternalInput")
    t["ident"] = nc.dram_tensor("ident", [128, 128], BF16,
                                kind="ExternalInput")
    t["triT"] = nc.dram_tensor("triT", [128, 128], BF16,
                               kind="ExternalInput")
    t["edgeT"] = nc.dram_tensor("edgeT", [128, 128], BF16,
                                kind="ExternalInput")
    t["edgegT"] = nc.dram_tensor("edgegT", [128, 128], BF16,
                                 kind="ExternalInput")
    t["y"] = nc.dram_tensor("y", [S, D], BF16, kind="ExternalOutput")

    with tile.TileContext(nc) as tc:
        t["tc"] = tc
        _emit(nc, t)
    nc.compile()
    _PROGRAM["nc"] = nc
    return nc


def _host_inputs(x, wq_w, wq_a, wq_b, wk_w, wk_a, wk_b, wv_w, wv_a, wv_b,
                 wo_w, wo_a, wo_b):
    f32 = np.float32
    bf16 = ml_dtypes.bfloat16
    Wq = (wq_w.astype(f32) + wq_b.astype(f32) @ wq_a.astype(f32))
    Wk = (wk_w.astype(f32) + wk_b.astype(f32) @ wk_a.astype(f32))
    Wv = (wv_w.astype(f32) + wv_b.astype(f32) @ wv_a.astype(f32))
    Wo = (wo_w.astype(f32) + wo_b.astype(f32) @ wo_a.astype(f32))

    perm = np.concatenate([np.arange(0, HD, 2), np.arange(1, HD, 2)])
    Wq_p = Wq.reshape(H, HD, D)[:, perm, :].reshape(H * HD, D)
    Wk_p = Wk.reshape(KVH, HD, D)[:, perm, :].reshape(KVH * HD, D)

    j = np.arange(HD // 2, dtype=np.float64)
    inv_freq = 1.0 / THETA ** (2.0 * j / HD)
    tpos = np.arange(S, dtype=np.float64)
    freqs = np.outer(inv_freq, tpos)                      # [64, S]
    cosT = np.cos(freqs)
    sinT = np.sin(freqs)
    cos2t = np.concatenate([cosT, cosT], 0).astype(bf16)
    sins2t = np.concatenate([-sinT, sinT], 0).astype(bf16)

    a = np.arange(128)
    triT = (a[:, None] <= a[None, :]).astype(bf16)
    edgeT = (a[:, None] > a[None, :]).astype(bf16)
    edgegT = ((a[:, None] > a[None, :]) | (a[:, None] < GLOBAL)).astype(bf16)
    ident = np.eye(128, dtype=bf16)

    common = dict(cos2t=cos2t, sins2t=sins2t, triT=triT, edgeT=edgeT,
                  edgegT=edgegT, ident=ident)

    def pack_w(wT, nf):
        # [D, nf] -> [128, 16, nf], partition-contiguous
        return np.ascontiguousarray(
            wT.reshape(16, 128, nf).transpose(1, 0, 2)).astype(bf16)

    NCH_ = S // TOK
    in_maps = []
    for b in range(B):
        xT = x[b].astype(f32).T.astype(bf16)            # [D, S]
        xh = np.ascontiguousarray(
            xT.reshape(16, 128, NCH_, TOK).transpose(2, 1, 0, 3))
        for g in range(KVH):
            woT = Wo[:, GF * g:GF * (g + 1)].T          # [GF, D]
            woh = np.ascontiguousarray(
                woT.reshape(GH, 128, D).transpose(1, 0, 2)).astype(bf16)
            in_maps.append(dict(
                xt=xh,
                wqt=pack_w(Wq_p[GF * g:GF * (g + 1), :].T, GF),
                wkt=pack_w(Wk_p[HD * g:HD * (g + 1), :].T, HD),
                wvt=pack_w(Wv[HD * g:HD * (g + 1), :].T, HD),
                wot=woh,
                **common,
            ))
    return in_maps


def kernel(**inputs):
    nc = _build_program()
    in_maps = _host_inputs(**inputs)
    res = None
    last_err = None
    for _attempt in range(3):
        try:
            res = run_bass_kernel_spmd(nc, in_maps,
                                       core_ids=list(range(NCORES)))
            break
        except Exception as e:  # transient first-exec device hiccups
            last_err = e
            import time as _time
            _time.sleep(2.0)
    if res is None:
        raise last_err
    out = np.zeros((B, S, D), dtype=np.float32)
    for b in range(B):
        for g in range(KVH):
            out[b] += res.results[b * KVH + g]["y"].astype(np.float32)
    return out


# revision 11
# speedup vs baseline: 1.0943x; 1.0943x over previous
"""Sparse GQA attention (causal + sliding window + global tokens) with LoRA
projections and RoPE, distributed over 8 TRN2 NeuronCores.

Sharding: batch (2) x kv-head-group (4). Core (b, g) computes q heads
4g..4g+3 and kv head g for batch b, producing a partial output-projection
sum; the host adds the 4 group partials per batch.

Host-side preprocessing (exact, linear):
  - LoRA folded into the dense weights: W_eff = W + B @ A.
  - Weights pre-transposed into matmul (lhsT / rhs) layouts, cast to bf16.
  - q/k weight rows permuted to the rotate-half layout (evens then odds)
    so RoPE becomes half-tile multiplies.
  - x transposed to [D, S] per batch (contraction dim on partitions).

Kernel structure (per core):
  Phase A (per 512-token chunk): QKV projections; RoPE on a bf16 SBUF
  copy of the projection PSUM; V transposed to natural [tok, hd] layout
  with a ones column appended (for softmax sums via matmul).
  Phase B (per 4-query-block quarter, interleaved with A): scores are
  computed TRANSPOSED (s^T[k, q]) batched per key-block covering its
  whole 5-query-block range in wide matmuls; exp on scalar engine;
  causal/window masking as post-exp {0,1} multiplies on DVE; P@V with
  the ones column yielding row sums in PSUM; normalization fused into
  the PSUM->SBUF copy; o transposed via one 128-col matmul; output
  projection per query block.
"""

import os
import sys

import numpy as np

for _p in ("/root/.axon_site", "/root/.axon_site/_ro/trn_rl_repo",
           "/root/.axon_site/_ro/pypackages", "/opt/trn_rl_repo"):
    if os.path.isdir(_p) and _p not in sys.path:
        sys.path.append(_p)

import ml_dtypes
import concourse.bacc as bacc
import concourse.mybir as mybir
import concourse.tile as tile
from concourse.bass_utils import run_bass_kernel_spmd

B, S, D = 2, 2048, 2048
H, KVH, HD = 16, 4, 128
WINDOW, GLOBAL = 512, 64
THETA = 1000000.0
NCORES = 8
GH = H // KVH          # q heads per core
GF = GH * HD           # 512 projection features per core
TOK = 512              # token chunk for projections
NCH = S // TOK         # 4 chunks
NQB = S // 128         # 16 query blocks
NKB = S // 128
INV_SQRT = 1.0 / float(np.sqrt(HD))

F32 = mybir.dt.float32
BF16 = mybir.dt.bfloat16

_PROGRAM = {}


def _emit(nc, t):
    """Emit the per-core Tile program. `t` maps input names to DRAM APs."""
    tc = t["tc"]
    from contextlib import ExitStack

    xTd = t["xt"].ap()      # [NCH,128,16,TOK] chunk-packed
    wqT = t["wqt"].ap()     # [128,16,GF]
    wkT = t["wkt"].ap()     # [128,16,HD]
    wvT = t["wvt"].ap()
    woT = t["wot"].ap()     # [128,GH,D]
    y = t["y"].ap()         # [S, D]

    with ExitStack() as stk:
        singles = stk.enter_context(tc.tile_pool(name="singles", bufs=1))
        persist = stk.enter_context(tc.tile_pool(name="persist", bufs=1))
        wpool = stk.enter_context(tc.tile_pool(name="wpool", bufs=1))
        xpool = stk.enter_context(tc.tile_pool(name="xpool", bufs=2))
        apool = stk.enter_context(tc.tile_pool(name="apool", bufs=2))
        bpool = stk.enter_context(tc.tile_pool(name="bpool", bufs=1))
        spool = stk.enter_context(tc.tile_pool(name="spool", bufs=4))
        cpool = stk.enter_context(tc.tile_pool(name="cpool", bufs=2))
        psa = stk.enter_context(tc.tile_pool(name="psa", bufs=1,
                                             space="PSUM"))
        psb = stk.enter_context(tc.tile_pool(name="psb", bufs=1,
                                             space="PSUM"))

        ident_sb = singles.tile([128, 128], BF16)
        nc.sync.dma_start(out=ident_sb, in_=t["ident"].ap())
        triT_sb = singles.tile([128, 128], BF16)
        nc.sync.dma_start(out=triT_sb, in_=t["triT"].ap())
        edgeT_sb = singles.tile([128, 128], BF16)
        nc.sync.dma_start(out=edgeT_sb, in_=t["edgeT"].ap())
        edgegT_sb = singles.tile([128, 128], BF16)
        nc.sync.dma_start(out=edgegT_sb, in_=t["edgegT"].ap())

        qT_sb = persist.tile([128, GH, S], BF16)    # [hd, head, tok]
        kT_sb = persist.tile([128, S], BF16)        # [hd, tok]
        v_sb = persist.tile([128, NKB, HD + 1], BF16)  # [tok%128, kblk, hd|1]
        nc.vector.memset(v_sb[:, :, HD:HD + 1], 1.0)

        # DMA order tuned for startup: x chunk 0 + small k/v weights first
        # so the first (k-head) matmuls start as early as possible.
        xt0 = xpool.tile([128, 16, TOK], BF16, tag="xt")
        nc.sync.dma_start(out=xt0[:, 0:8, :], in_=xTd[0][:, 0:8, :])
        wk_sb = wpool.tile([128, 16, HD], BF16)
        nc.sync.dma_start(out=wk_sb, in_=wkT)
        wv_sb = wpool.tile([128, 16, HD], BF16)
        nc.sync.dma_start(out=wv_sb, in_=wvT)
        nc.sync.dma_start(out=xt0[:, 8:16, :], in_=xTd[0][:, 8:16, :])
        wq_sb = wpool.tile([128, 16, GF], BF16)
        nc.sync.dma_start(out=wq_sb[:, 0:8, :], in_=wqT[:, 0:8, :])
        nc.sync.dma_start(out=wq_sb[:, 8:16, :], in_=wqT[:, 8:16, :])
        cos_sb = wpool.tile([128, S], BF16)
        nc.sync.dma_start(out=cos_sb, in_=t["cos2t"].ap())
        sin_sb = wpool.tile([128, S], BF16)
        nc.sync.dma_start(out=sin_sb, in_=t["sins2t"].ap())
        wo_sb = wpool.tile([128, GH, D], BF16)

        ptk_tiles = {}
        ptg_tiles = {}

        def emit_chunk(c):
            cs = slice(c * TOK, (c + 1) * TOK)
            if c == 0:
                xt = xt0
            else:
                xt = xpool.tile([128, 16, TOK], BF16, tag="xt")
                nc.sync.dma_start(out=xt, in_=xTd[c])
            # v first, in natural [tok, hd] layout directly (operands
            # swapped: lhsT = x token-slices), then k, then q0..q3.
            # k/q get rotate-half RoPE; k is needed first by the
            # following attention quarter's score matmuls.
            vnat = psa.tile([128, TOK], F32, tag="pq", bufs=2)
            for b2 in range(TOK // 128):
                bs = slice(b2 * 128, (b2 + 1) * 128)
                for a in range(16):
                    nc.tensor.matmul(vnat[:, bs], xt[:, a, bs],
                                     wv_sb[:, a, :],
                                     start=(a == 0), stop=(a == 15))
            for b2 in range(TOK // 128):
                bs = slice(b2 * 128, (b2 + 1) * 128)
                nc.vector.tensor_copy(v_sb[:, c * 4 + b2, 0:HD],
                                      vnat[:, bs])
            for h in [GH] + list(range(GH)):
                if h == GH:
                    wslc = wk_sb
                    dst = kT_sb[:, cs]
                else:
                    wslc = wq_sb[:, :, h * HD:(h + 1) * HD]
                    dst = qT_sb[:, h, cs]
                pq = psa.tile([128, TOK], F32, tag="pq", bufs=2)
                for a in range(16):
                    nc.tensor.matmul(pq, wslc[:, a, :], xt[:, a, :],
                                     start=(a == 0), stop=(a == 15))
                # rotate-half RoPE on DVE; cross-partition reads are
                # legal because the rotated input comes from PSUM
                t1 = apool.tile([128, TOK], BF16, tag="t1", bufs=2)
                nc.vector.tensor_mul(t1[0:64, :], pq[64:128, :],
                                     sin_sb[0:64, cs])
                nc.vector.tensor_mul(t1[64:128, :], pq[0:64, :],
                                     sin_sb[64:128, cs])
                t2 = apool.tile([128, TOK], BF16, tag="t2", bufs=2)
                nc.vector.tensor_mul(t2, pq, cos_sb[:, cs])
                nc.vector.tensor_add(dst, t2, t1)

        def emit_quarter(qtr):
            qis = list(range(4 * (qtr - 1), 4 * qtr))
            for h in range(GH):
                for kb in qis:
                    # score tile kb covers q-blocks kb..kb+4 transposed:
                    # s^T[k, q]; diag mask at slice 0, window edge at 4
                    nq = min(5, NQB - kb)
                    W = 128 * nq
                    k0 = kb * 128
                    ps = psb.tile([128, 640], F32, tag="ps", bufs=2)
                    if W > 512:
                        nc.tensor.matmul(ps[:, 0:512],
                                         kT_sb[:, k0:k0 + 128],
                                         qT_sb[:, h, k0:k0 + 512],
                                         start=True, stop=True)
                        nc.tensor.matmul(ps[:, 512:W],
                                         kT_sb[:, k0:k0 + 128],
                                         qT_sb[:, h, k0 + 512:k0 + W],
                                         start=True, stop=True)
                    else:
                        nc.tensor.matmul(ps[:, 0:W],
                                         kT_sb[:, k0:k0 + 128],
                                         qT_sb[:, h, k0:k0 + W],
                                         start=True, stop=True)
                    ptk = bpool.tile([128, 640], BF16, tag=f"ptk{h}",
                                     bufs=8)
                    nc.scalar.activation(ptk[:, 0:W], ps[:, 0:W],
                                         mybir.ActivationFunctionType.Exp,
                                         scale=INV_SQRT)
                    nc.vector.tensor_mul(ptk[:, 0:128], ptk[:, 0:128],
                                         triT_sb)
                    if nq == 5:
                        nc.vector.tensor_mul(
                            ptk[:, 512:640], ptk[:, 512:640],
                            edgegT_sb if kb == 0 else edgeT_sb)
                    ptk_tiles[(h, kb)] = ptk
                if qtr >= 2:
                    q0b = max(5, 4 * (qtr - 1))
                    Wg = (4 * qtr - q0b) * 128
                    q0 = q0b * 128
                    gps = psb.tile([128, 640], F32, tag="ps", bufs=2)
                    nc.tensor.matmul(gps[0:64, 0:Wg], kT_sb[:, 0:64],
                                     qT_sb[:, h, q0:q0 + Wg],
                                     start=True, stop=True)
                    ptg = bpool.tile([128, 512], BF16, tag=f"ptg{h}",
                                     bufs=2)
                    nc.scalar.activation(ptg[0:64, 0:Wg], gps[0:64, 0:Wg],
                                         mybir.ActivationFunctionType.Exp,
                                         scale=INV_SQRT)
                    ptg_tiles[h] = (ptg, q0)
            for qi in qis:
                ot = cpool.tile([128, GH, 128], BF16, tag="ot", bufs=2)
                for h in range(GH):
                    # po tile layout: [0:129] P@V + sums, [132:260] o^T
                    po = psb.tile([128, 260], F32, tag="po", bufs=2)
                    kbs = list(range(max(0, qi - 4), qi + 1))
                    n_mm = len(kbs) + (1 if qi >= 5 else 0)
                    for i, kb in enumerate(kbs):
                        j = qi - kb
                        nc.tensor.matmul(
                            po[:, 0:HD + 1],
                            ptk_tiles[(h, kb)][:, j * 128:(j + 1) * 128],
                            v_sb[:, kb, 0:HD + 1],
                            start=(i == 0), stop=(i == n_mm - 1))
                    if qi >= 5:
                        ptg, q0 = ptg_tiles[h]
                        off = qi * 128 - q0
                        nc.tensor.matmul(po[:, 0:HD + 1],
                                         ptg[0:64, off:off + 128],
                                         v_sb[0:64, 0, 0:HD + 1],
                                         start=False, stop=True)
                    inv = spool.tile([128, 1], F32, tag="inv")
                    nc.vector.reciprocal(inv, po[:, HD:HD + 1])
                    onat = spool.tile([128, 128], BF16, tag="onat")
                    nc.vector.tensor_scalar_mul(onat, po[:, 0:HD], inv)
                    nc.tensor.matmul(po[:, 132:260], onat, ident_sb,
                                     start=True, stop=True)
                    nc.scalar.copy(ot[:, h, :], po[:, 132:260])
                # output projection for this token block
                ysb = cpool.tile([128, D], BF16, tag="ysb", bufs=2)
                for cchunk in range(4):
                    ns = slice(cchunk * 512, (cchunk + 1) * 512)
                    py = psb.tile([128, 640], F32, tag="ps", bufs=2)
                    for hh in range(GH):
                        nc.tensor.matmul(py[:, 0:512], ot[:, hh, :],
                                         wo_sb[:, hh, ns],
                                         start=(hh == 0),
                                         stop=(hh == GH - 1))
                    if cchunk % 2 == 0:
                        nc.scalar.copy(ysb[:, ns], py[:, 0:512])
                    else:
                        nc.vector.tensor_copy(ysb[:, ns], py[:, 0:512])
                nc.sync.dma_start(out=y[qi * 128:(qi + 1) * 128, :],
                                  in_=ysb)

        emit_chunk(0)
        emit_chunk(1)
        nc.sync.dma_start(out=wo_sb, in_=woT)
        emit_quarter(1)
        emit_chunk(2)
        emit_quarter(2)
        emit_chunk(3)
        emit_quarter(3)
        emit_quarter(4)


def _build_program():
    if "nc" in _PROGRAM:
        return _PROGRAM["nc"]
    nc = bacc.Bacc("TRN2", target_bir_lowering=False, debug=False,
                   num_devices=NCORES)
    t = {}
    t["xt"] = nc.dram_tensor("xt", [NCH, 128, 16, TOK], BF16,
                             kind="ExternalInput")
    t["wqt"] = nc.dram_tensor("wqt", [128, 16, GF], BF16,
                              kind="ExternalInput")
    t["wkt"] = nc.dram_tensor("wkt", [128, 16, HD], BF16,
                              kind="ExternalInput")
    t["wvt"] = nc.dram_tensor("wvt", [128, 16, HD], BF16,
                              kind="ExternalInput")
    t["wot"] = nc.dram_tensor("wot", [128, GH, D], BF16,
                              kind="ExternalInput")
    t["cos2t"] = nc.dram_tensor("cos2t", [128, S], BF16,
                                kind="ExternalInput")
    t["sins2t"] = nc.dram_tensor("sins2t", [128, S], BF16,
                                 kind="ExternalInput")
    t["ident"] = nc.dram_tensor("ident", [128, 128], BF16,
                                kind="ExternalInput")
    t["triT"] = nc.dram_tensor("triT", [128, 128], BF16,
                               kind="ExternalInput")
    t["edgeT"] = nc.dram_tensor("edgeT", [128, 128], BF16,
                                kind="ExternalInput")
    t["edgegT"] = nc.dram_tensor("edgegT", [128, 128], BF16,
                                 kind="ExternalInput")
    t["y"] = nc.dram_tensor("y", [S, D], BF16, kind="ExternalOutput")

    with tile.TileContext(nc) as tc:
        t["tc"] = tc
        _emit(nc, t)
    nc.compile()
    _PROGRAM["nc"] = nc
    return nc


def _host_inputs(x, wq_w, wq_a, wq_b, wk_w, wk_a, wk_b, wv_w, wv_a, wv_b,
                 wo_w, wo_a, wo_b):
    f32 = np.float32
    bf16 = ml_dtypes.bfloat16
    Wq = (wq_w.astype(f32) + wq_b.astype(f32) @ wq_a.astype(f32))
    Wk = (wk_w.astype(f32) + wk_b.astype(f32) @ wk_a.astype(f32))
    Wv = (wv_w.astype(f32) + wv_b.astype(f32) @ wv_a.astype(f32))
    Wo = (wo_w.astype(f32) + wo_b.astype(f32) @ wo_a.astype(f32))

    perm = np.concatenate([np.arange(0, HD, 2), np.arange(1, HD, 2)])
    Wq_p = Wq.reshape(H, HD, D)[:, perm, :].reshape(H * HD, D)
    Wk_p = Wk.reshape(KVH, HD, D)[:, perm, :].reshape(KVH * HD, D)

    j = np.arange(HD // 2, dtype=np.float64)
    inv_freq = 1.0 / THETA ** (2.0 * j / HD)
    tpos = np.arange(S, dtype=np.float64)
    freqs = np.outer(inv_freq, tpos)                      # [64, S]
    cosT = np.cos(freqs)
    sinT = np.sin(freqs)
    cos2t = np.concatenate([cosT, cosT], 0).astype(bf16)
    sins2t = np.concatenate([-sinT, sinT], 0).astype(bf16)

    a = np.arange(128)
    triT = (a[:, None] <= a[None, :]).astype(bf16)
    edgeT = (a[:, None] > a[None, :]).astype(bf16)
    edgegT = ((a[:, None] > a[None, :]) | (a[:, None] < GLOBAL)).astype(bf16)
    ident = np.eye(128, dtype=bf16)

    common = dict(cos2t=cos2t, sins2t=sins2t, triT=triT, edgeT=edgeT,
                  edgegT=edgegT, ident=ident)

    def pack_w(wT, nf):
        # [D, nf] -> [128, 16, nf], partition-contiguous
        return np.ascontiguousarray(
            wT.reshape(16, 128, nf).transpose(1, 0, 2)).astype(bf16)

    NCH_ = S // TOK
    in_maps = []
    for b in range(B):
        xT = x[b].astype(f32).T.astype(bf16)            # [D, S]
        xh = np.ascontiguousarray(
            xT.reshape(16, 128, NCH_, TOK).transpose(2, 1, 0, 3))
        for g in range(KVH):
            woT = Wo[:, GF * g:GF * (g + 1)].T          # [GF, D]
            woh = np.ascontiguousarray(
                woT.reshape(GH, 128, D).transpose(1, 0, 2)).astype(bf16)
            in_maps.append(dict(
                xt=xh,
                wqt=pack_w(Wq_p[GF * g:GF * (g + 1), :].T, GF),
                wkt=pack_w(Wk_p[HD * g:HD * (g + 1), :].T, HD),
                wvt=pack_w(Wv[HD * g:HD * (g + 1), :].T, HD),
                wot=woh,
                **common,
            ))
    return in_maps


def kernel(**inputs):
    nc = _build_program()
    in_maps = _host_inputs(**inputs)
    res = None
    last_err = None
    for _attempt in range(3):
        try:
            res = run_bass_kernel_spmd(nc, in_maps,
                                       core_ids=list(range(NCORES)))
            break
        except Exception as e:  # transient first-exec device hiccups
            last_err = e
            import time as _time
            _time.sleep(2.0)
    if res is None:
        raise last_err
    out = np.zeros((B, S, D), dtype=np.float32)
    for b in range(B):
        for g in range(KVH):
            out[b] += res.results[b * KVH + g]["y"].astype(np.float32)
    return out


# revision 15
# speedup vs baseline: 1.2776x; 1.1675x over previous
"""Sparse GQA attention (causal + sliding window + global tokens) with LoRA
projections and RoPE, distributed over 8 TRN2 NeuronCores.

Sharding: batch (2) x kv-head-group (4). Core (b, g) computes q heads
4g..4g+3 and kv head g for batch b, producing a partial output-projection
sum; the host adds the 4 group partials per batch.

Host-side preprocessing (exact, linear):
  - LoRA folded into the dense weights: W_eff = W + B @ A.
  - Weights pre-transposed into matmul (lhsT / rhs) layouts, cast to bf16.
  - q/k weight rows permuted to the rotate-half layout (evens then odds)
    so RoPE becomes half-tile multiplies.
  - x transposed to [D, S] per batch (contraction dim on partitions).

Kernel structure (per core):
  Phase A (per 512-token chunk): QKV projections; RoPE on a bf16 SBUF
  copy of the projection PSUM; V transposed to natural [tok, hd] layout
  with a ones column appended (for softmax sums via matmul).
  Phase B (per 4-query-block quarter, interleaved with A): scores are
  computed TRANSPOSED (s^T[k, q]) batched per key-block covering its
  whole 5-query-block range in wide matmuls; exp on scalar engine;
  causal/window masking as post-exp {0,1} multiplies on DVE; P@V with
  the ones column yielding row sums in PSUM; normalization fused into
  the PSUM->SBUF copy; o transposed via one 128-col matmul; output
  projection per query block.
"""

import os
import sys

import numpy as np

for _p in ("/root/.axon_site", "/root/.axon_site/_ro/trn_rl_repo",
           "/root/.axon_site/_ro/pypackages", "/opt/trn_rl_repo"):
    if os.path.isdir(_p) and _p not in sys.path:
        sys.path.append(_p)

import ml_dtypes
import concourse.bacc as bacc
import concourse.mybir as mybir
import concourse.tile as tile
from concourse.bass_utils import run_bass_kernel_spmd

B, S, D = 2, 2048, 2048
H, KVH, HD = 16, 4, 128
WINDOW, GLOBAL = 512, 64
THETA = 1000000.0
NCORES = 8
GH = H // KVH          # q heads per core
GF = GH * HD           # 512 projection features per core
TOK = 512              # token chunk for projections
NCH = S // TOK         # 4 chunks
NQB = S // 128         # 16 query blocks
NKB = S // 128
INV_SQRT = 1.0 / float(np.sqrt(HD))

F32 = mybir.dt.float32
BF16 = mybir.dt.bfloat16

_PROGRAM = {}


def _emit(nc, t):
    """Emit the per-core Tile program. `t` maps input names to DRAM APs."""
    tc = t["tc"]
    from contextlib import ExitStack

    xTd = t["xt"].ap()      # [NCH,128,16,TOK] chunk-packed
    wqT = t["wqt"].ap()     # [128,16,GF]
    wkT = t["wkt"].ap()     # [128,16,HD]
    wvT = t["wvt"].ap()
    woT = t["wot"].ap()     # [128,GH,D]
    y = t["y"].ap()         # [S, D]

    with ExitStack() as stk:
        singles = stk.enter_context(tc.tile_pool(name="singles", bufs=1))
        persist = stk.enter_context(tc.tile_pool(name="persist", bufs=1))
        wpool = stk.enter_context(tc.tile_pool(name="wpool", bufs=1))
        xpool = stk.enter_context(tc.tile_pool(name="xpool", bufs=2))
        apool = stk.enter_context(tc.tile_pool(name="apool", bufs=2))
        bpool = stk.enter_context(tc.tile_pool(name="bpool", bufs=1))
        spool = stk.enter_context(tc.tile_pool(name="spool", bufs=4))
        cpool = stk.enter_context(tc.tile_pool(name="cpool", bufs=2))
        psa = stk.enter_context(tc.tile_pool(name="psa", bufs=1,
                                             space="PSUM"))
        psb = stk.enter_context(tc.tile_pool(name="psb", bufs=1,
                                             space="PSUM"))

        ident_sb = singles.tile([128, 128], BF16)
        nc.sync.dma_start(out=ident_sb, in_=t["ident"].ap())
        triT_sb = singles.tile([128, 128], BF16)
        nc.sync.dma_start(out=triT_sb, in_=t["triT"].ap())
        edgeT_sb = singles.tile([128, 128], BF16)
        nc.sync.dma_start(out=edgeT_sb, in_=t["edgeT"].ap())
        edgegT_sb = singles.tile([128, 128], BF16)
        nc.sync.dma_start(out=edgegT_sb, in_=t["edgegT"].ap())

        qT_sb = persist.tile([128, GH, S], BF16)    # [hd, head, tok]
        kT_sb = persist.tile([128, S], BF16)        # [hd, tok]
        v_sb = persist.tile([128, NKB, HD + 1], BF16)  # [tok%128, kblk, hd|1]
        nc.vector.memset(v_sb[:, :, HD:HD + 1], 1.0)

        # DMA order tuned for startup: x chunk 0 + small k/v weights first
        # so the first (k-head) matmuls start as early as possible.
        xt0 = xpool.tile([128, 16, TOK], BF16, tag="xt")
        nc.sync.dma_start(out=xt0[:, 0:8, :], in_=xTd[0][:, 0:8, :])
        wk_sb = wpool.tile([128, 16, HD], BF16)
        nc.sync.dma_start(out=wk_sb, in_=wkT)
        wv_sb = wpool.tile([128, 16, HD], BF16)
        nc.sync.dma_start(out=wv_sb, in_=wvT)
        nc.sync.dma_start(out=xt0[:, 8:16, :], in_=xTd[0][:, 8:16, :])
        wq_sb = wpool.tile([128, 16, GF], BF16)
        nc.sync.dma_start(out=wq_sb[:, 0:8, :], in_=wqT[:, 0:8, :])
        nc.sync.dma_start(out=wq_sb[:, 8:16, :], in_=wqT[:, 8:16, :])
        cos_sb = wpool.tile([128, S], BF16)
        nc.sync.dma_start(out=cos_sb, in_=t["cos2t"].ap())
        sin_sb = wpool.tile([128, S], BF16)
        nc.sync.dma_start(out=sin_sb, in_=t["sins2t"].ap())
        wo_sb = wpool.tile([128, GH, D], BF16)

        ptk_tiles = {}
        ptg_tiles = {}

        def emit_chunk(c):
            cs = slice(c * TOK, (c + 1) * TOK)
            if c == 0:
                xt = xt0
            else:
                xt = xpool.tile([128, 16, TOK], BF16, tag="xt")
                nc.sync.dma_start(out=xt, in_=xTd[c])
            # v first, in natural [tok, hd] layout directly (operands
            # swapped: lhsT = x token-slices), then k, then q0..q3.
            # k/q get rotate-half RoPE; k is needed first by the
            # following attention quarter's score matmuls.
            vnat = psa.tile([128, TOK], F32, tag="pq", bufs=2)
            for b2 in range(TOK // 128):
                bs = slice(b2 * 128, (b2 + 1) * 128)
                for a in range(16):
                    nc.tensor.matmul(vnat[:, bs], xt[:, a, bs],
                                     wv_sb[:, a, :],
                                     start=(a == 0), stop=(a == 15))
            for b2 in range(TOK // 128):
                bs = slice(b2 * 128, (b2 + 1) * 128)
                nc.vector.tensor_copy(v_sb[:, c * 4 + b2, 0:HD],
                                      vnat[:, bs])
            for h in [GH] + list(range(GH)):
                if h == GH:
                    wslc = wk_sb
                    dst = kT_sb[:, cs]
                else:
                    wslc = wq_sb[:, :, h * HD:(h + 1) * HD]
                    dst = qT_sb[:, h, cs]
                pq = psa.tile([128, TOK], F32, tag="pq", bufs=2)
                for a in range(16):
                    nc.tensor.matmul(pq, wslc[:, a, :], xt[:, a, :],
                                     start=(a == 0), stop=(a == 15))
                # rotate-half RoPE on DVE; cross-partition reads are
                # legal because the rotated input comes from PSUM
                t1 = apool.tile([128, TOK], BF16, tag="t1", bufs=2)
                nc.vector.tensor_mul(t1[0:64, :], pq[64:128, :],
                                     sin_sb[0:64, cs])
                nc.vector.tensor_mul(t1[64:128, :], pq[0:64, :],
                                     sin_sb[64:128, cs])
                t2 = apool.tile([128, TOK], BF16, tag="t2", bufs=2)
                nc.vector.tensor_mul(t2, pq, cos_sb[:, cs])
                nc.vector.tensor_add(dst, t2, t1)

        def emit_scores(qtr):
            qis = list(range(4 * (qtr - 1), 4 * qtr))
            for h in range(GH):
                for kb in qis:
                    # score tile kb covers q-blocks kb..kb+4 transposed:
                    # s^T[k, q]; diag mask at slice 0, window edge at 4
                    nq = min(5, NQB - kb)
                    W = 128 * nq
                    k0 = kb * 128
                    ps = psb.tile([128, 640], F32, tag="ps", bufs=2)
                    if W > 512:
                        nc.tensor.matmul(ps[:, 0:512],
                                         kT_sb[:, k0:k0 + 128],
                                         qT_sb[:, h, k0:k0 + 512],
                                         start=True, stop=True)
                        nc.tensor.matmul(ps[:, 512:W],
                                         kT_sb[:, k0:k0 + 128],
                                         qT_sb[:, h, k0 + 512:k0 + W],
                                         start=True, stop=True)
                    else:
                        nc.tensor.matmul(ps[:, 0:W],
                                         kT_sb[:, k0:k0 + 128],
                                         qT_sb[:, h, k0:k0 + W],
                                         start=True, stop=True)
                    ptk = bpool.tile([128, 640], BF16, tag=f"ptk{h}",
                                     bufs=8)
                    nc.scalar.activation(ptk[:, 0:W], ps[:, 0:W],
                                         mybir.ActivationFunctionType.Exp,
                                         scale=INV_SQRT)
                    nc.vector.tensor_mul(ptk[:, 0:128], ptk[:, 0:128],
                                         triT_sb)
                    if nq == 5:
                        nc.vector.tensor_mul(
                            ptk[:, 512:640], ptk[:, 512:640],
                            edgegT_sb if kb == 0 else edgeT_sb)
                    ptk_tiles[(h, kb)] = ptk
                if qtr >= 2:
                    q0b = max(5, 4 * (qtr - 1))
                    Wg = (4 * qtr - q0b) * 128
                    q0 = q0b * 128
                    gps = psb.tile([128, 640], F32, tag="ps", bufs=2)
                    nc.tensor.matmul(gps[0:64, 0:Wg], kT_sb[:, 0:64],
                                     qT_sb[:, h, q0:q0 + Wg],
                                     start=True, stop=True)
                    ptg = bpool.tile([128, 512], BF16, tag=f"ptg{h}",
                                     bufs=2)
                    nc.scalar.activation(ptg[0:64, 0:Wg], gps[0:64, 0:Wg],
                                         mybir.ActivationFunctionType.Exp,
                                         scale=INV_SQRT)
                    ptg_tiles[h] = (ptg, q0)

        def emit_pv(qtr):
            qis = list(range(4 * (qtr - 1), 4 * qtr))
            for qi in qis:
                ot = cpool.tile([128, GH, 128], BF16, tag="ot", bufs=2)
                for h in range(GH):
                    # po tile layout: [0:129] P@V + sums, [132:260] o^T
                    po = psb.tile([128, 260], F32, tag="po", bufs=2)
                    kbs = list(range(max(0, qi - 4), qi + 1))
                    n_mm = len(kbs) + (1 if qi >= 5 else 0)
                    for i, kb in enumerate(kbs):
                        j = qi - kb
                        nc.tensor.matmul(
                            po[:, 0:HD + 1],
                            ptk_tiles[(h, kb)][:, j * 128:(j + 1) * 128],
                            v_sb[:, kb, 0:HD + 1],
                            start=(i == 0), stop=(i == n_mm - 1))
                    if qi >= 5:
                        ptg, q0 = ptg_tiles[h]
                        off = qi * 128 - q0
                        nc.tensor.matmul(po[:, 0:HD + 1],
                                         ptg[0:64, off:off + 128],
                                         v_sb[0:64, 0, 0:HD + 1],
                                         start=False, stop=True)
                    inv = spool.tile([128, 1], F32, tag="inv")
                    nc.vector.reciprocal(inv, po[:, HD:HD + 1])
                    onat = spool.tile([128, 128], BF16, tag="onat")
                    nc.vector.tensor_scalar_mul(onat, po[:, 0:HD], inv)
                    nc.tensor.matmul(po[:, 132:260], onat, ident_sb,
                                     start=True, stop=True)
                    nc.scalar.copy(ot[:, h, :], po[:, 132:260])
                # output projection for this token block; DMA out each
                # 512-col piece as soon as it is copied (short drain)
                ysb = cpool.tile([128, D], BF16, tag="ysb", bufs=2)
                for cchunk in range(4):
                    ns = slice(cchunk * 512, (cchunk + 1) * 512)
                    py = psb.tile([128, 640], F32, tag="ps", bufs=2)
                    for hh in range(GH):
                        nc.tensor.matmul(py[:, 0:512], ot[:, hh, :],
                                         wo_sb[:, hh, ns],
                                         start=(hh == 0),
                                         stop=(hh == GH - 1))
                    if cchunk % 2 == 0:
                        nc.scalar.copy(ysb[:, ns], py[:, 0:512])
                    else:
                        nc.vector.tensor_copy(ysb[:, ns], py[:, 0:512])
                    nc.sync.dma_start(
                        out=y[qi * 128:(qi + 1) * 128, ns],
                        in_=ysb[:, ns])

        # Pipeline: score batches (KB) are emitted well after the chunk
        # whose RoPE output they read, and PV batches read only score
        # tiles from a previous KB step — the PE never waits on the
        # DVE RoPE or scalar exp chains.
        emit_chunk(0)
        emit_chunk(1)
        nc.sync.dma_start(out=wo_sb, in_=woT)
        emit_chunk(2)
        emit_scores(1)
        emit_pv(1)
        emit_scores(2)
        emit_chunk(3)
        emit_pv(2)
        emit_scores(3)
        emit_pv(3)
        emit_scores(4)
        emit_pv(4)


def _build_program():
    if "nc" in _PROGRAM:
        return _PROGRAM["nc"]
    nc = bacc.Bacc("TRN2", target_bir_lowering=False, debug=False,
                   num_devices=NCORES)
    t = {}
    t["xt"] = nc.dram_tensor("xt", [NCH, 128, 16, TOK], BF16,
                             kind="ExternalInput")
    t["wqt"] = nc.dram_tensor("wqt", [128, 16, GF], BF16,
                              kind="ExternalInput")
    t["wkt"] = nc.dram_tensor("wkt", [128, 16, HD], BF16,
                              kind="ExternalInput")
    t["wvt"] = nc.dram_tensor("wvt", [128, 16, HD], BF16,
                              kind="ExternalInput")
    t["wot"] = nc.dram_tensor("wot", [128, GH, D], BF16,
                              kind="ExternalInput")
    t["cos2t"] = nc.dram_tensor("cos2t", [128, S], BF16,
                                kind="ExternalInput")
    t["sins2t"] = nc.dram_tensor("sins2t", [128, S], BF16,
                                 kind="ExternalInput")
    t["ident"] = nc.dram_tensor("ident", [128, 128], BF16,
                                kind="ExternalInput")
    t["triT"] = nc.dram_tensor("triT", [128, 128], BF16,
                               kind="ExternalInput")
    t["edgeT"] = nc.dram_tensor("edgeT", [128, 128], BF16,
                                kind="ExternalInput")
    t["edgegT"] = nc.dram_tensor("edgegT", [128, 128], BF16,
                                 kind="ExternalInput")
    t["y"] = nc.dram_tensor("y", [S, D], BF16, kind="ExternalOutput")

    with tile.TileContext(nc) as tc:
        t["tc"] = tc
        _emit(nc, t)
    nc.compile()
    _PROGRAM["nc"] = nc
    return nc


def _host_inputs(x, wq_w, wq_a, wq_b, wk_w, wk_a, wk_b, wv_w, wv_a, wv_b,
                 wo_w, wo_a, wo_b):
    f32 = np.float32
    bf16 = ml_dtypes.bfloat16
    Wq = (wq_w.astype(f32) + wq_b.astype(f32) @ wq_a.astype(f32))
    Wk = (wk_w.astype(f32) + wk_b.astype(f32) @ wk_a.astype(f32))
    Wv = (wv_w.astype(f32) + wv_b.astype(f32) @ wv_a.astype(f32))
    Wo = (wo_w.astype(f32) + wo_b.astype(f32) @ wo_a.astype(f32))

    perm = np.concatenate([np.arange(0, HD, 2), np.arange(1, HD, 2)])
    Wq_p = Wq.reshape(H, HD, D)[:, perm, :].reshape(H * HD, D)
    Wk_p = Wk.reshape(KVH, HD, D)[:, perm, :].reshape(KVH * HD, D)

    j = np.arange(HD // 2, dtype=np.float64)
    inv_freq = 1.0 / THETA ** (2.0 * j / HD)
    tpos = np.arange(S, dtype=np.float64)
    freqs = np.outer(inv_freq, tpos)                      # [64, S]
    cosT = np.cos(freqs)
    sinT = np.sin(freqs)
    cos2t = np.concatenate([cosT, cosT], 0).astype(bf16)
    sins2t = np.concatenate([-sinT, sinT], 0).astype(bf16)

    a = np.arange(128)
    triT = (a[:, None] <= a[None, :]).astype(bf16)
    edgeT = (a[:, None] > a[None, :]).astype(bf16)
    edgegT = ((a[:, None] > a[None, :]) | (a[:, None] < GLOBAL)).astype(bf16)
    ident = np.eye(128, dtype=bf16)

    common = dict(cos2t=cos2t, sins2t=sins2t, triT=triT, edgeT=edgeT,
                  edgegT=edgegT, ident=ident)

    def pack_w(wT, nf):
        # [D, nf] -> [128, 16, nf], partition-contiguous
        return np.ascontiguousarray(
            wT.reshape(16, 128, nf).transpose(1, 0, 2)).astype(bf16)

    NCH_ = S // TOK
    in_maps = []
    for b in range(B):
        xT = x[b].astype(f32).T.astype(bf16)            # [D, S]
        xh = np.ascontiguousarray(
            xT.reshape(16, 128, NCH_, TOK).transpose(2, 1, 0, 3))
        for g in range(KVH):
            woT = Wo[:, GF * g:GF * (g + 1)].T          # [GF, D]
            woh = np.ascontiguousarray(
                woT.reshape(GH, 128, D).transpose(1, 0, 2)).astype(bf16)
            in_maps.append(dict(
                xt=xh,
                wqt=pack_w(Wq_p[GF * g:GF * (g + 1), :].T, GF),
                wkt=pack_w(Wk_p[HD * g:HD * (g + 1), :].T, HD),
                wvt=pack_w(Wv[HD * g:HD * (g + 1), :].T, HD),
                wot=woh,
                **common,
            ))
    return in_maps


def kernel(**inputs):
    nc = _build_program()
    in_maps = _host_inputs(**inputs)
    res = None
    last_err = None
    for _attempt in range(3):
        try:
            res = run_bass_kernel_spmd(nc, in_maps,
                                       core_ids=list(range(NCORES)))
            break
        except Exception as e:  # transient first-exec device hiccups
            last_err = e
            import time as _time
            _time.sleep(2.0)
    if res is None:
        raise last_err
    out = np.zeros((B, S, D), dtype=np.float32)
    for b in range(B):
        for g in range(KVH):
            out[b] += res.results[b * KVH + g]["y"].astype(np.float32)
    return out
